# revision 1
# baseline (speedup 1.0000x reference)
"""3-layer GAT forward for nn_GAT_21045339750566 on 8 TRN2 NeuronCores.

Self-contained: host-side edge preprocessing (dst-shard + window sort +
int16 gather-index packing), bass/tile kernel build, execution via
concourse run_bass_kernel_spmd, and output reassembly.

Hardcoded problem shape: N=50000 nodes, E=800000 edges, F=256, H=4 heads,
D=64, C=40 classes, 8 cores.
"""
import os
import sys
import numpy as np

sys.path.insert(0, '/opt/trn_rl_repo')

"""
# Walrus on this stack rejects instructions carrying more than MAX_WAITS sem
waits. Post-pass: hoist excess waits onto InstNoOp instructions inserted just
before the offending instruction (same engine, program order preserved)."""

from concourse import mybir

MAX_WAITS = 1


def legalize_waits(nc, max_waits=MAX_WAITS):
    n_fixed = 0
    for fn in nc.m.functions:
        for blk in fn.blocks:
            il = blk.instructions
            i = 0
            while i < len(il):
                inst = il[i]
                si = inst.sync_info
                if si is not None and len(si.on_wait) > max_waits:
                    waits = list(si.on_wait)
                    keep = waits[-max_waits:]
                    extra = waits[:-max_waits]
                    inst.sync_info = mybir.SyncInfo(
                        on_wait=keep, on_update=list(si.on_update)
                    )
                    nops = []
                    for j in range(0, len(extra), max_waits):
                        nop = mybir.InstNoOp(
                            name=nc.get_next_instruction_name(),
                            engine=inst.engine,
                            bass_nofuse=True,
                            sync_info=mybir.SyncInfo(
                                on_wait=extra[j : j + max_waits], on_update=[]
                            ),
                        )
                        try:
                            nc.register_instruction(nop)
                        except Exception:
                            pass
                        nops.append(nop)
                    for k, nop in enumerate(nops):
                        il.insert(i + k, nop)
                    i += len(nops)
                    n_fixed += 1
                i += 1
    return n_fixed


import numpy as np
import concourse.bass as bass
import concourse.mybir as mybir
import concourse.tile as tile
from concourse import library_config
from concourse.library_overlay import lower_extended_insts

F32 = mybir.dt.float32
I16 = mybir.dt.int16
AF = mybir.ActivationFunctionType
OP = mybir.AluOpType
AX = mybir.AxisListType

DUMMY = 200.0
MAXG = 2048   # max idxs per dma_gather
WIN = 128
NEG_SLOPE = 0.2


class Meta:
    pass


def build_meta(src, dst, N, n_cores, split):
    """SPMD-uniform per-core edge metadata. Per-core edge order: windows
    ascending; within a window group A (src<split) then group B, each padded
    to a multiple of 128 with dummy edges (idx 0, dstloc=DUMMY)."""
    shard = N // n_cores
    nwin = (shard + WIN - 1) // WIN
    m = Meta()
    shard_pad = nwin * WIN
    m.N, m.n_cores, m.shard, m.nwin, m.split = N, n_cores, shard, nwin, split
    m.shard_pad = shard_pad
    m.N_pad = n_cores * shard_pad
    # padded global ids: node n -> core(n)*shard_pad + (n % shard)
    src = (src // shard) * shard_pad + (src % shard)

    pcw = []
    for c in range(n_cores):
        sel = (dst // shard) == c
        s_c, d_c = src[sel], dst[sel]
        dloc = (d_c - c * shard).astype(np.int64)
        order = np.argsort(dloc, kind='stable')
        s_c, dloc = s_c[order], dloc[order]
        wins = []
        for w in range(nwin):
            lo, hi = np.searchsorted(dloc, [w * WIN, (w + 1) * WIN])
            sw, dw = s_c[lo:hi], dloc[lo:hi] - w * WIN
            a = sw < split
            wins.append((sw[a], sw[~a] - split, dw[a], dw[~a]))
        pcw.append(wins)

    up = lambda n: max(-(-n // 128) * 128, 0)
    nA = [max(128, max(up(len(pcw[c][w][0])) for c in range(n_cores))) for w in range(nwin)]
    nB = [max(up(len(pcw[c][w][1])) for c in range(n_cores)) for w in range(nwin)]

    m.win_desc = []
    icol = chcol = 0
    for w in range(nwin):
        m.win_desc.append(dict(nA=nA[w], nB=nB[w], offA=icol, offB=icol + nA[w] // 16,
                               choff=chcol))
        icol += (nA[w] + nB[w]) // 16
        chcol += (nA[w] + nB[w]) // 128
    m.tot_icols, m.tot_chcols = icol, chcol
    m.maxE = max(nA[w] + nB[w] for w in range(nwin))
    m.max_chunks = m.maxE // 128
    m.tot_rows = sum(nA) + sum(nB)

    def wrap16(idx):
        return np.tile(idx.reshape(-1, 16).T, (8, 1))

    m.idx16, m.dstrow, m.dstcolT = [], [], []
    for c in range(n_cores):
        i16 = np.zeros((128, m.tot_icols), np.int16)
        drow = np.full((nwin, m.maxE), DUMMY, np.float32)
        dcolT = np.full((128, max(m.tot_chcols, 1)), DUMMY, np.float32)
        for w in range(nwin):
            sA, sB, dA, dB = pcw[c][w]
            d = m.win_desc[w]
            a = np.zeros(d['nA'], np.int64); a[:len(sA)] = sA
            b = np.zeros(d['nB'], np.int64); b[:len(sB)] = sB
            i16[:, d['offA']:d['offA'] + d['nA'] // 16] = wrap16(a)
            if d['nB']:
                i16[:, d['offB']:d['offB'] + d['nB'] // 16] = wrap16(b)
            dl = np.full(d['nA'] + d['nB'], DUMMY, np.float32)
            dl[:len(dA)] = dA
            dl[d['nA']:d['nA'] + len(dB)] = dB
            drow[w, :len(dl)] = dl
            dcolT[:, d['choff']:d['choff'] + len(dl) // 128] = dl.reshape(-1, 128).T
        m.idx16.append(i16); m.dstrow.append(drow); m.dstcolT.append(dcolT)
    return m


def blockdiag_host(al, heads, dim):
    """al [heads, dim] -> [heads*dim, heads] block-diagonal placement."""
    out = np.zeros((heads * dim, heads), np.float32)
    for h in range(heads):
        out[h * dim:(h + 1) * dim, h] = al[h]
    return out


def gather_plan(d, split):
    """-> list of (cnt, idx_col_off, chunk_off, base) per window descriptor."""
    plan, ch = [], 0
    for cnt, off, base in ((d['nA'], d['offA'], 0), (d['nB'], d['offB'], split)):
        done = 0
        while done < cnt:
            step = min(MAXG, cnt - done)
            plan.append((step, off + done // 16, ch, base))
            done += step
            ch += step // 128
    return plan


def bcast_cols(t_ap, off, stride, count, width):
    """AP over SBUF tile row-slice: free pattern [(stride,count),(0,width)]
    starting at free-elem `off` (per-partition)."""
    base = t_ap[:, off:off + 1]
    return bass.AP(base.tensor, base.offset, [base.ap[0], [stride, count], [0, width]])


def build_kernel(nc, meta, F, H, Dh, C):
    N, shard, nwin, split = meta.N_pad, meta.shard_pad, meta.nwin, meta.split
    nblk = F // 128
    ntile = nwin
    T3W = 64
    assert C + 1 <= T3W

    io = {}
    def inp(name, shape, dtype=F32):
        io[name] = nc.dram_tensor(name, shape, dtype, kind="ExternalInput")
        return io[name]

    X = inp("x", [N, F])
    XOWN = inp("x_own", [shard, F])  # N/shard already padded
    W1 = inp("W1", [F, F]); W2 = inp("W2", [F, F]); W3 = inp("W3", [F, C])
    B1 = inp("b1", [1, F]); B2 = inp("b2", [1, F]); B3 = inp("b3", [1, C])
    ALM1 = inp("alm1", [F, 2 * H]); ALM2 = inp("alm2", [F, 2 * H])
    ALM3 = inp("alm3", [C, 2])
    IDX = inp("idx16", [128, meta.tot_icols], I16)
    DROW = inp("dstrow", [nwin, meta.maxE])
    DCOLT = inp("dstcolT", [128, max(meta.tot_chcols, 1)])
    IOTC = inp("iota_col", [128, 1])
    IOTR = inp("iota_row", [1, 128])
    IOTARR = inp("iota_rows", [128, 128])
    ONESR = inp("ones_row", [1, 128])
    ONESC = inp("ones_col", [128, 1])
    BLKM = inp("blkmask", [H, F])
    ONESH = inp("ones_hcol", [H, 128])
    IDENT = inp("ident", [128, 128])
    OUT = nc.dram_tensor("out", [shard, C], F32, kind="ExternalOutput")

    T2W = 320
    x2_shard = nc.dram_tensor("x2_shard", [shard, T2W], F32)
    x2_full = nc.dram_tensor("x2_full", [N, T2W], F32, addr_space="Shared")
    t3_shard = nc.dram_tensor("t3_shard", [shard, T3W], F32)
    t3_full = nc.dram_tensor("t3_full", [N, T3W], F32, addr_space="Shared")
    er_tab = nc.dram_tensor("er_tab", [shard, H], F32)
    er3_tab = nc.dram_tensor("er3_tab", [shard, 1], F32)
    import os as _os
    _edbg = _os.environ.get("GAT_EDGE_DEBUG") == "1"
    if _edbg:
        DBG_XG = nc.dram_tensor("dbg_xg", [128, 40 * F], F32, kind="ExternalOutput")
        DBG_EL = nc.dram_tensor("dbg_el", [128, 512], F32, kind="ExternalOutput")
        DBG_SC = nc.dram_tensor("dbg_sc", [128, 512], F32, kind="ExternalOutput")
        DBG_MT = nc.dram_tensor("dbg_mt", [128, 4096], F32, kind="ExternalOutput")
        DBG_PS = nc.dram_tensor("dbg_ps", [128, 512], F32, kind="ExternalOutput")
        io['_edbg'] = True

    reg_cache = {}
    def reg(v):
        if v not in reg_cache:
            reg_cache[v] = nc.gpsimd.to_reg(v)
        return reg_cache[v]

    with tile.TileContext(nc) as tc:
        with tc.tile_pool(name="cst", bufs=1) as cst:

            nc.gpsimd.load_library(library_config.mlp)

            def load_const(name, shape, dtype=F32, rearr=None):
                tl = cst.tile(shape, dtype, tag=name)
                if rearr:
                    # chunked [A*128, W] -> tile [128, A*W]
                    w = io[name].shape[1]
                    for a in range(io[name].shape[0] // 128):
                        nc.sync.dma_start(out=tl[:, a * w:(a + 1) * w],
                                          in_=io[name][a * 128:(a + 1) * 128, :])
                else:
                    nc.sync.dma_start(out=tl[:], in_=io[name][:])
                return tl

            ident = load_const("ident", [128, 128])
            iotc = load_const("iota_col", [128, 1])
            iotr = load_const("iota_row", [1, 128])
            iotarr = load_const("iota_rows", [128, 128])
            onesr = load_const("ones_row", [1, 128])
            onesc = load_const("ones_col", [128, 1])
            blkmask = load_const("blkmask", [H, F])
            ones_hcol = load_const("ones_hcol", [H, 128])
            idx_sb = load_const("idx16", [128, meta.tot_icols], I16)
            dcolT = load_const("dstcolT", [128, max(meta.tot_chcols, 1)])
            w1_sb = load_const("W1", [128, nblk * F], rearr="(a p) f -> p (a f)")
            w2_sb = load_const("W2", [128, nblk * F], rearr="(a p) f -> p (a f)")
            w3_sb = load_const("W3", [128, nblk * C], rearr="(a p) f -> p (a f)")
            b1_sb = load_const("b1", [1, F])
            b2_sb = load_const("b2", [1, F])
            b3_sb = load_const("b3", [1, C])
            alm1_sb = load_const("alm1", [128, nblk * 2 * H], rearr="(a p) f -> p (a f)")
            alm2_sb = load_const("alm2", [128, nblk * 2 * H], rearr="(a p) f -> p (a f)")
            alm3_sb = load_const("alm3", [C, 2])

            setup_ctx = tc.tile_pool(name="setup_ps", bufs=1, space="PSUM")
            pst = setup_ctx.__enter__()
            def bcast_row(src_ap, width, pool, ppool, tag):
                out_t = pool.tile([128, width], F32, tag=tag)
                for c0 in range(0, width, 512):
                    cw = min(512, width - c0)
                    pb = ppool.tile([128, 512], F32, tag="brps")
                    nc.tensor.matmul(out=pb[:, :cw], lhsT=onesr[:],
                                     rhs=src_ap[:, c0:c0 + cw], start=True, stop=True)
                    nc.scalar.copy(out=out_t[:, c0:c0 + cw], in_=pb[:, :cw])
                return out_t

            # ---- transpose W blocks; fold V in row + column layouts
            def wT_blocks(w_sb, tag):
                """-> sbuf tile [128, nblk*nblk*128]; block (a,k) at
                [:, (a*nblk+k)*128 ...] = W[a-chunk fin, k-chunk fout].T"""
                wt = cst.tile([128, nblk * nblk * 128], F32, tag=tag)
                for a in range(nblk):
                    for k in range(nblk):
                        pT = pst.tile([128, 128], F32, tag="psT")
                        nc.tensor.transpose(
                            out=pT[:], in_=w_sb[:, a * F + k * 128: a * F + k * 128 + 128],
                            identity=ident[:])
                        nc.scalar.copy(out=wt[:, (a * nblk + k) * 128:(a * nblk + k + 1) * 128],
                                       in_=pT[:])
                return wt

            def fold_v(wt, alm_sb, w2h, tag):
                """-> (v_col [128, nblk*w2h]  (chunk a = V[fin_a, :]),
                       v_row [w2h, F])"""
                v_col = cst.tile([128, nblk * w2h], F32, tag=f"vc{tag}")
                v_row = cst.tile([w2h, F], F32, tag=f"vr{tag}")
                for a in range(nblk):
                    pc = pst.tile([128, w2h], F32, tag="psVc")
                    pr = pst.tile([w2h, 128], F32, tag="psVr")
                    for k in range(nblk):
                        blk = wt[:, (a * nblk + k) * 128:(a * nblk + k + 1) * 128]
                        nc.tensor.matmul(out=pc[:], lhsT=blk,
                                         rhs=alm_sb[:, k * w2h:(k + 1) * w2h],
                                         start=(k == 0), stop=(k == nblk - 1))
                        nc.tensor.matmul(out=pr[:], lhsT=alm_sb[:, k * w2h:(k + 1) * w2h],
                                         rhs=blk, start=(k == 0), stop=(k == nblk - 1))
                    nc.vector.tensor_copy(out=v_col[:, a * w2h:(a + 1) * w2h], in_=pc[:])
                    nc.vector.tensor_copy(out=v_row[:, a * 128:(a + 1) * 128], in_=pr[:])
                return v_col, v_row

            wt1 = wT_blocks(w1_sb, "wt")
            v1_col, v1_row = fold_v(wt1, alm1_sb, 2 * H, "1")
            wt2 = wT_blocks(w2_sb, "wt")
            v2_col, v2_row = fold_v(wt2, alm2_sb, 2 * H, "2")

            # vflat[0, f] = V_el[f, head(f)]
            def vflat_of(v_row, tag):
                # vflat_rep[p, f] = V_el[f, head(f)], replicated across partitions
                vrm = cst.tile([H, F], F32, tag=tag + "m")
                nc.vector.tensor_tensor(out=vrm[:], in0=v_row[0:H, :], in1=blkmask[:],
                                        op=OP.mult)
                pv = pst.tile([128, F], F32, tag="brps2")
                nc.tensor.matmul(out=pv[:], lhsT=ones_hcol[:], rhs=vrm[:],
                                 start=True, stop=True)
                vf = cst.tile([128, F], F32, tag=tag)
                nc.scalar.copy(out=vf[:], in_=pv[:])
                return vf

            b1_rep = bcast_row(b1_sb[:], F, cst, pst, "b1r")
            b2_rep = bcast_row(b2_sb[:], F, cst, pst, "b2r")
            b3_rep = bcast_row(b3_sb[:], C, cst, pst, "b3r")

            # v3_col [128, nblk*2] ; w3v combined rhs [128, nblk*(C+2)]
            w3T = cst.tile([C, nblk * 128], F32, tag="w3T")
            for a in range(nblk):
                pT = pst.tile([128, 128], F32, tag="psT")
                nc.tensor.transpose(out=pT[:C, :], in_=w3_sb[:, a * C:(a + 1) * C],
                                    identity=ident[:])
                nc.scalar.copy(out=w3T[:, a * 128:(a + 1) * 128], in_=pT[:C, :])
            w3v = cst.tile([128, nblk * (C + 2)], F32, tag="w3v")
            for a in range(nblk):
                pv = pst.tile([128, 2], F32, tag="psV3")
                nc.tensor.matmul(out=pv[:], lhsT=w3T[:, a * 128:(a + 1) * 128],
                                 rhs=alm3_sb[:], start=True, stop=True)
                nc.vector.tensor_copy(out=w3v[:, a * (C + 2) + C: (a + 1) * (C + 2)], in_=pv[:])
                nc.vector.tensor_copy(out=w3v[:, a * (C + 2): a * (C + 2) + C],
                                      in_=w3_sb[:, a * C:(a + 1) * C])

            setup_ctx.__exit__(None, None, None)

            # ---- helper: transpose a [128, F] sbuf tile into F/128 blocks
            def transpose_tile(src_ap, pool, ppool):
                xT = pool.tile([128, F], F32, tag="xT")
                for k in range(nblk):
                    pT = ppool.tile([128, 128], F32, tag="psT2")
                    nc.tensor.transpose(out=pT[:], in_=src_ap[:, k * 128:(k + 1) * 128],
                                        identity=ident[:])
                    nc.scalar.copy(out=xT[:, k * 128:(k + 1) * 128], in_=pT[:])
                return xT

            # ---- er1 table from XOWN
            with tc.tile_pool(name="erp", bufs=3) as erp, \
                 tc.tile_pool(name="erpp", bufs=2, space="PSUM") as erpp:
                for it in range(ntile):
                    r0 = it * 128
                    xt = erp.tile([128, F], F32, tag="erx")
                    nc.sync.dma_start(out=xt[:], in_=XOWN[r0:r0 + 128, :])
                    xT = transpose_tile(xt[:], erp, erpp)
                    pe = erpp.tile([128, H], F32, tag="psER")
                    for k in range(nblk):
                        nc.tensor.matmul(out=pe[:],
                                         lhsT=xT[:, k * 128:(k + 1) * 128],
                                         rhs=v1_col[:, k * 2 * H + H: (k + 1) * 2 * H],
                                         start=(k == 0), stop=(k == nblk - 1))
                    ero = erp.tile([128, H], F32, tag="ero")
                    nc.vector.tensor_copy(out=ero[:], in_=pe[:])
                    nc.sync.dma_start(out=er_tab[r0:r0 + 128, :], in_=ero[:])

            # ================= edge phase =================
            def edge_phase(table, tblw, feats, heads, vcol, finalize):
                with tc.tile_pool(name="exg", bufs=2) as gp, \
                     tc.tile_pool(name="emm", bufs=3) as mp, \
                     tc.tile_pool(name="ewk", bufs=2) as wp, \
                     tc.tile_pool(name="ep1", bufs=1, space="PSUM") as p1, \
                     tc.tile_pool(name="ep2", bufs=2, space="PSUM") as p2:
                    for w in range(nwin):
                        d = meta.win_desc[w]
                        nE = d['nA'] + d['nB']
                        nch = nE // 128
                        r0 = w * WIN
                        xg = gp.tile([128, meta.max_chunks * tblw], F32, tag="xg")
                        xg3 = xg[:].rearrange("p (c r) -> p c r", r=tblw)
                        for (cnt, coff, ch0, base) in gather_plan(d, split):
                            src_ap = table[0:split, :] if base == 0 else table[split:, :]
                            nc.gpsimd.dma_gather(
                                out_ap=xg3[:, ch0:ch0 + cnt // 128, :],
                                in_ap=src_ap,
                                idxs_ap=idx_sb[:, coff:coff + cnt // 16],
                                num_idxs=cnt, num_idxs_reg=reg(cnt),
                                elem_size=tblw, single_packet=False)
                        drow = wp.tile([1, meta.maxE], F32, tag="drow")
                        nc.sync.dma_start(out=drow[:, :nE], in_=DROW[w:w + 1, :nE])
                        erw = wp.tile([128, heads], F32, tag="erw")
                        er_src = er_tab if heads > 1 else er3_tab
                        nc.sync.dma_start(out=erw[:], in_=er_src[r0:r0 + 128, :])
                        # Mt[j, e] one-hot (via row-replicated dstloc)
                        drep = mp.tile([128, meta.maxE], F32, tag="drep")
                        for c0 in range(0, nE, 512):
                            cw = min(512, nE - c0)
                            pb = p2.tile([128, 512], F32, tag="pmisc")
                            nc.tensor.matmul(out=pb[:, :cw], lhsT=onesr[:],
                                             rhs=drow[:, c0:c0 + cw], start=True, stop=True)
                            nc.scalar.copy(out=drep[:, c0:c0 + cw], in_=pb[:, :cw])
                        mt = mp.tile([128, meta.maxE], F32, tag="mt")
                        nc.vector.tensor_tensor(
                            out=mt[:, :nE], in0=iotc[:].to_broadcast([128, nE]),
                            in1=drep[:, :nE], op=OP.is_equal)
                        # pscore: region0 = er[dstloc[e],h]; region1 = el[e,h]
                        NS = heads * nch
                        pscore = p1.tile([128, 2 * NS], F32, tag="pscore")
                        for c in range(nch):
                            nc.tensor.matmul(out=pscore[:, c * heads:(c + 1) * heads],
                                             lhsT=mt[:, c * 128:(c + 1) * 128],
                                             rhs=erw[:], start=(c == 0),
                                             stop=(vcol is None and c == nch - 1),
                                             skip_group_check=True)
                            if vcol is not None:
                                xgTc = wp.tile([128, F], F32, tag="xgT")
                                for k in range(nblk):
                                    pT = p2.tile([128, 512], F32, tag="pmisc")
                                    nc.tensor.transpose(
                                        out=pT[:, :128],
                                        in_=xg3[:, c, k * 128:(k + 1) * 128],
                                        identity=ident[:])
                                    nc.scalar.copy(out=xgTc[:, k * 128:(k + 1) * 128],
                                                   in_=pT[:, :128])
                                for k in range(nblk):
                                    nc.tensor.matmul(
                                        out=pscore[:, NS + c * heads: NS + (c + 1) * heads],
                                        lhsT=xgTc[:, k * 128:(k + 1) * 128],
                                        rhs=vcol[:, k * 2 * heads: k * 2 * heads + heads],
                                        start=False,
                                        stop=(c == nch - 1 and k == nblk - 1),
                                        skip_group_check=True)
                        # scores: ex = exp(lrelu(el + er))
                        sco = wp.tile([128, heads * nch], F32, tag="sco")
                        if vcol is not None:
                            nc.scalar.copy(out=sco[:], in_=pscore[:, NS:2 * NS])
                            nc.vector.tensor_add(out=sco[:], in0=sco[:],
                                                 in1=pscore[:, 0:NS])
                        else:
                            nc.vector.tensor_add(
                                out=sco[:].rearrange("p (c a) -> p c a", a=heads)[:, 0:nch, :],
                                in0=pscore[:, 0:NS].rearrange("p (c a) -> p c a", a=heads)[:, 0:nch, :],
                                in1=xg3[:, 0:nch, feats:feats + heads])
                        nc.vector.scalar_tensor_tensor(out=sco[:], in0=sco[:],
                                                       scalar=NEG_SLOPE, in1=sco[:],
                                                       op0=OP.mult, op1=OP.max)
                        nc.scalar.activation(out=sco[:], in_=sco[:], func=AF.Exp)
                        # per-head aggregation: M'[e,j] = (dstloc==j)*ex ; agg_h = M'^T @ [x|1]
                        paggs = [p1.tile([128, feats + 1], F32, tag=f"pagg{h}",
                                         name=f"pagg{h}_{w}")
                                 for h in range(heads)]
                        for c in range(nch):
                            for h in range(heads):
                                mpr = mp.tile([128, 128], F32, tag="mp")
                                nc.vector.scalar_tensor_tensor(
                                    out=mpr[:], in0=iotarr[:],
                                    scalar=dcolT[:, d['choff'] + c: d['choff'] + c + 1],
                                    in1=sco[:, c * heads + h: c * heads + h + 1].to_broadcast([128, 128]),
                                    op0=OP.is_equal, op1=OP.mult)
                                nc.tensor.matmul(out=paggs[h][:, 0:feats], lhsT=mpr[:],
                                                 rhs=xg3[:, c, 0:feats],
                                                 start=(c == 0), stop=False,
                                                 skip_group_check=True)
                                nc.tensor.matmul(out=paggs[h][:, feats:feats + 1],
                                                 lhsT=mpr[:], rhs=onesc[:],
                                                 start=False, stop=(c == nch - 1),
                                                 skip_group_check=True)
                        esr = wp.tile([128, heads], F32, tag="esr")
                        for h in range(heads):
                            nc.vector.tensor_scalar_max(
                                out=esr[:, h:h + 1], in0=paggs[h][:, feats:feats + 1],
                                scalar1=1e-30)
                        nc.vector.reciprocal(out=esr[:], in_=esr[:])
                        if _edbg and w == 0 and heads > 1:
                            dps = wp.tile([128, heads * nch], F32, tag="dps")
                            nc.vector.tensor_copy(out=dps[:], in_=pscore[:, NS:2 * NS])
                            nc.sync.dma_start(out=DBG_EL[:, :heads * nch], in_=dps[:])
                            dpe = wp.tile([128, heads * nch], F32, tag="dpe")
                            nc.vector.tensor_copy(out=dpe[:], in_=pscore[:, 0:NS])
                            nc.sync.dma_start(out=DBG_MT[:, :heads * nch], in_=dpe[:])
                            dsc = wp.tile([128, heads * nch], F32, tag="dsc")
                            nc.vector.tensor_copy(out=dsc[:], in_=sco[:])
                            nc.sync.dma_start(out=DBG_SC[:, :heads * nch], in_=dsc[:])
                            dp0 = wp.tile([128, F + 1], F32, tag="dp0")
                            nc.vector.tensor_copy(out=dp0[:], in_=paggs[0][:])
                            nc.sync.dma_start(out=DBG_PS[:, :F + 1], in_=dp0[:])
                        finalize(w, r0, paggs, esr, wp, p1, p2)

            # fused finalize for L1/L2: z = sum_h scaled_agg_h @ W[:, hblock];
            # x_next = elu(z + b); then next-layer er (or L3 table) from x_next.
            def make_fin12(w_sb_, b_rep_, ver_col, x_dst, l3_tail):
                def fin(w, r0, paggs, esr, wp, p1, p2):
                    z = p1.tile([128, F], F32, tag="pz")
                    for h in range(H):
                        ow = wp.tile([128, F], F32, tag="ow")
                        nc.scalar.activation(out=ow[:], in_=paggs[h][:, 0:F],
                                             func=AF.Copy, scale=esr[:, h:h + 1])
                        for k in range(nblk):
                            pT = p2.tile([128, 512], F32, tag="pmisc")
                            nc.tensor.transpose(out=pT[:, :128],
                                                in_=ow[:, k * 128:(k + 1) * 128],
                                                identity=ident[:])
                            owT = wp.tile([128, 128], F32, tag="owT")
                            nc.scalar.copy(out=owT[:], in_=pT[:, :128])
                            nc.tensor.matmul(
                                out=z[:, h * Dh:(h + 1) * Dh], lhsT=owT[:],
                                rhs=w_sb_[:, k * F + h * Dh: k * F + h * Dh + Dh],
                                start=(h == 0 and k == 0),
                                stop=(h == H - 1 and k == nblk - 1),
                                skip_group_check=True)
                    zb = wp.tile([128, F], F32, tag="zb")
                    nc.vector.tensor_add(out=zb[:], in0=z[:], in1=b_rep_[:])
                    e0 = wp.tile([128, F], F32, tag="e0")
                    nc.vector.tensor_scalar_min(out=e0[:], in0=zb[:], scalar1=0.0)
                    nc.scalar.activation(out=e0[:], in_=e0[:], func=AF.Exp)
                    nc.vector.tensor_scalar_add(out=e0[:], in0=e0[:], scalar1=-1.0)
                    xn = wp.tile([128, F], F32, tag="xn")
                    nc.vector.scalar_tensor_tensor(out=xn[:], in0=zb[:], scalar=0.0,
                                                   in1=e0[:], op0=OP.max, op1=OP.add)
                    if x_dst is not None:
                        nc.sync.dma_start(out=x_dst[r0:r0 + 128, 0:F], in_=xn[:])
                    xnT = wp.tile([128, F], F32, tag="xnT")
                    for k in range(nblk):
                        pT = p2.tile([128, 512], F32, tag="pmisc")
                        nc.tensor.transpose(out=pT[:, :128],
                                            in_=xn[:, k * 128:(k + 1) * 128],
                                            identity=ident[:])
                        nc.scalar.copy(out=xnT[:, k * 128:(k + 1) * 128], in_=pT[:, :128])
                    if not l3_tail:
                        pe = p1.tile([128, 2 * H], F32, tag="pz")
                        for k in range(nblk):
                            nc.tensor.matmul(out=pe[:],
                                             lhsT=xnT[:, k * 128:(k + 1) * 128],
                                             rhs=ver_col[:, k * 2 * H:(k + 1) * 2 * H],
                                             start=(k == 0), stop=(k == nblk - 1))
                        ero = wp.tile([128, H], F32, tag="ero2")
                        nc.vector.tensor_copy(out=ero[:], in_=pe[:, H:2 * H])
                        nc.sync.dma_start(out=er_tab[r0:r0 + 128, :], in_=ero[:])
                        elo = wp.tile([128, 64], F32, tag="elo2")
                        nc.vector.memset(elo[:, H:], 0.0)
                        nc.vector.tensor_copy(out=elo[:, :H], in_=pe[:, 0:H])
                        nc.sync.dma_start(out=x_dst[r0:r0 + 128, F:F + 64],
                                          in_=elo[:])
                    else:
                        pe = p1.tile([128, C + 2], F32, tag="pz")
                        for k in range(nblk):
                            nc.tensor.matmul(out=pe[:],
                                             lhsT=xnT[:, k * 128:(k + 1) * 128],
                                             rhs=w3v[:, k * (C + 2):(k + 1) * (C + 2)],
                                             start=(k == 0), stop=(k == nblk - 1))
                        t3o = wp.tile([128, T3W], F32, tag="t3o")
                        nc.vector.memset(t3o[:, C + 1:], 0.0)
                        nc.vector.tensor_copy(out=t3o[:, :C + 1], in_=pe[:, :C + 1])
                        nc.sync.dma_start(out=t3_shard[r0:r0 + 128, :], in_=t3o[:])
                        er3o = wp.tile([128, 1], F32, tag="er3o")
                        nc.vector.tensor_copy(out=er3o[:], in_=pe[:, C + 1:C + 2])
                        nc.sync.dma_start(out=er3_tab[r0:r0 + 128, :], in_=er3o[:])
                return fin

            import os
            _dbg = os.environ.get("GAT_DEBUG_STAGE", "")
            _dbg_stop = _dbg in ("0", "1")

            edge_phase(X, F, F, H, v1_col,
                       make_fin12(w1_sb, b1_rep, v2_col, x2_shard, l3_tail=False))

            if not _dbg_stop:
                nc.gpsimd.collective_compute(
                    "AllGather", OP.bypass, replica_groups=[list(range(meta.n_cores))],
                    ins=[x2_shard[:]], outs=[x2_full[:]])

                # ---- L2 (finalize computes L3 table directly)
                edge_phase(x2_full, 320, F, H, None,
                           make_fin12(w2_sb, b2_rep, None, None, l3_tail=True))

                nc.gpsimd.collective_compute(
                    "AllGather", OP.bypass, replica_groups=[list(range(meta.n_cores))],
                    ins=[t3_shard[:]], outs=[t3_full[:]])

                # ---- L3
                def fin3(w, r0, paggs, esr, wp, p1, p2):
                    pagg = paggs[0]
                    ow = wp.tile([128, C], F32, tag="ow3")
                    nc.scalar.activation(out=ow[:], in_=pagg[:, :C], func=AF.Copy,
                                         scale=esr[:, 0:1])
                    nc.vector.tensor_add(out=ow[:], in0=ow[:], in1=b3_rep[:])
                    negmax = wp.tile([128, 1], F32, tag="nm")
                    nc.vector.tensor_reduce(out=negmax[:], in_=ow[:], axis=AX.X,
                                            op=OP.max, negate=True)
                    ex = wp.tile([128, C], F32, tag="lex")
                    sume = wp.tile([128, 1], F32, tag="se")
                    nc.scalar.activation(out=ex[:], in_=ow[:], func=AF.Exp,
                                         bias=negmax[:], accum_out=sume[:])
                    lns = wp.tile([128, 1], F32, tag="ln")
                    nc.scalar.activation(out=lns[:], in_=sume[:], func=AF.Ln)
                    adj = wp.tile([128, 1], F32, tag="adj")
                    nc.vector.tensor_tensor(out=adj[:], in0=negmax[:], in1=lns[:],
                                            op=OP.subtract)
                    res = wp.tile([128, C], F32, tag="res")
                    nc.vector.tensor_scalar_add(out=res[:], in0=ow[:], scalar1=adj[:])
                    nc.sync.dma_start(out=OUT[r0:r0 + 128, :], in_=res[:])

                edge_phase(t3_full, T3W, C, 1, None, fin3)

    lower_extended_insts(nc)
    return io


def prepare_inputs(inputs, meta, F, H, Dh, C, core):
    """Per-core in_map from full inputs + meta."""
    N, shard = meta.N, meta.shard
    sp = meta.shard_pad
    x = np.asarray(inputs['x'], np.float32)
    xpad = np.zeros((meta.N_pad, F), np.float32)
    for cc in range(meta.n_cores):
        xpad[cc * sp: cc * sp + shard] = x[cc * shard:(cc + 1) * shard]
    m = {
        'x': xpad,
        'x_own': np.ascontiguousarray(xpad[core * sp:(core + 1) * sp]),
        'W1': np.asarray(inputs['W1'], np.float32),
        'W2': np.asarray(inputs['W2'], np.float32),
        'W3': np.asarray(inputs['W3'], np.float32),
        'b1': np.asarray(inputs['b1'], np.float32).reshape(1, F),
        'b2': np.asarray(inputs['b2'], np.float32).reshape(1, F),
        'b3': np.asarray(inputs['b3'], np.float32).reshape(1, C),
        'alm1': np.concatenate([blockdiag_host(np.asarray(inputs['al1'], np.float32), H, Dh),
                                blockdiag_host(np.asarray(inputs['ar1'], np.float32), H, Dh)], 1),
        'alm2': np.concatenate([blockdiag_host(np.asarray(inputs['al2'], np.float32), H, Dh),
                                blockdiag_host(np.asarray(inputs['ar2'], np.float32), H, Dh)], 1),
        'alm3': np.concatenate([np.asarray(inputs['al3'], np.float32).reshape(C, 1),
                                np.asarray(inputs['ar3'], np.float32).reshape(C, 1)], 1),
        'idx16': meta.idx16[core],
        'dstrow': meta.dstrow[core],
        'dstcolT': meta.dstcolT[core],
        'iota_col': np.arange(128, dtype=np.float32).reshape(128, 1),
        'iota_row': np.arange(128, dtype=np.float32).reshape(1, 128),
        'iota_rows': np.tile(np.arange(128, dtype=np.float32), (128, 1)),
        'ones_row': np.ones((1, 128), np.float32),
        'ones_col': np.ones((128, 1), np.float32),
        'blkmask': np.kron(np.eye(H, dtype=np.float32), np.ones((1, Dh), np.float32)),
        'ones_hcol': np.ones((H, 128), np.float32),
        'ident': np.eye(128, dtype=np.float32),
    }
    return m


_CACHE = {}


def kernel(**inputs):
    import concourse.bass as bass
    from concourse.bass_utils import run_bass_kernel_spmd

    N, F, H, Dh, C, NCORES, SPLIT = 50000, 256, 4, 64, 40, 8, 32768
    ei = np.asarray(inputs["edge_index"])
    src = ei[0].astype(np.int64)
    dst = ei[1].astype(np.int64)

    key = "k"
    if key not in _CACHE:
        meta = build_meta(src.copy(), dst, N, NCORES, SPLIT)
        nc = bass.Bass("TRN2", target_bir_lowering=False, debug=False,
                       num_devices=NCORES)
        build_kernel(nc, meta, F, H, Dh, C)
        legalize_waits(nc)
        _CACHE[key] = (meta, nc)
    meta, nc = _CACHE[key]

    in_maps = [prepare_inputs(inputs, meta, F, H, Dh, C, c) for c in range(NCORES)]
    trace = os.environ.get("GAT_TRACE") == "1"
    kw = {}
    if trace:
        kw = dict(trace=True, tmpdir=os.environ.get("GAT_TRACE_DIR",
                                                    "/tmp/gat_trace"))
    res = run_bass_kernel_spmd(nc, in_maps, list(range(NCORES)), **kw)
    if trace and res.exec_time_ns is not None:
        print(f"HW exec time: {res.exec_time_ns} ns")
    sh = meta.shard
    out = np.concatenate([res.results[c]["out"][:sh] for c in range(NCORES)], 0)
    return out.astype(np.float32)

